# revision 7
# baseline (speedup 1.0000x reference)
"""Trainium2 Bass kernel for DifferentiableMemory (B=8, S=4096, H=1024, M=1024).

Data-parallel over batch: one batch per NeuronCore x 8 cores, weights
replicated. All on-device activations live feature-on-partition ("T layout"),
produced by free host-side transposes, so no on-device transposes are needed:

  per core (batch b), with xT = hidden[b].T as [H, S] bf16:
    KmemT = WkT-proj(initT)     initT = xT[:, idx] (static gather, host-side)
    kn    = KmemT * rsqrt(colsum(KmemT^2))      (PE ones-reduce over partitions)
    V2    = (Vmem-proj)^T @ Wo2^T               [m, h'] - folds retrieved@Wo2^T
                                                into attn @ V2 (saves 6.4 GF)
    per 512-col chunk of s:
      QT    = WqT-proj(xT chunk); qn = QT * rsqrt(colsum(QT^2))
      simT  = kn^T-matmul(qn)    [m, s]  (cosine sims, bounded -> exp w/o max)
      expT  = Exp(simT); attnT = expT * recip(ones-reduce(expT))
      outT  = Wo1-proj(xT chunk) + V2-matmul(attnT) + bo   (single PSUM chain)

Matmuls in bf16 (fp32 PSUM accumulation); output returned fp32.
"""
import sys

sys.path.insert(0, "/opt/trn_rl_repo")

import numpy as np
import ml_dtypes

BF16 = ml_dtypes.bfloat16

B, S, H, M = 8, 4096, 1024, 1024
N_CORES = 8
P = 128          # partitions
KT = H // P      # 8 feature tiles
SC = 512         # s-chunk (PSUM bank = 512 fp32)
NCH = S // SC    # 8 chunks
MT = M // P      # 8 memory tiles

_cache = {}


def _idx():
    """Replicate reference: jnp.linspace(0.0, s-1, M).astype(int32).
    Computed with in-process jax so platform-specific fp32 rounding matches
    the grader's reference; numpy fallback differs in at most a few slots."""
    if "idx" in _cache:
        return _cache["idx"]
    try:
        import jax.numpy as jnp

        idx = np.asarray(jnp.linspace(0.0, S - 1, M).astype(jnp.int32))
    except Exception:
        idx = np.linspace(0.0, S - 1, M).astype(np.float32).astype(np.int32)
    _cache["idx"] = idx
    return idx


def _split_excess_waits(nc, mybir):
    """This container's walrus accepts at most 1 sem-wait per instruction
    (setupSyncWait raises "Too many sync wait commands" beyond that), while
    Tile's add_semaphores freely attaches several. Move excess waits onto
    preceding same-engine NoOps - engine streams execute in order, so a wait
    on an earlier instruction gates everything after it."""
    n = 0
    for f in nc.m.functions:
        for bb in f.blocks:
            insts = list(bb.instructions)
            new = []
            changed = False
            for inst in insts:
                si = getattr(inst, "sync_info", None)
                waits = list(si.on_wait) if si is not None and si.on_wait else []
                if len(waits) > 1:
                    for w in waits[:-1]:
                        new.append(
                            mybir.InstNoOp(
                                name=f"{inst.name}-wsplit{n}",
                                engine=inst.engine,
                                sync_info=mybir.SyncInfo(on_wait=[w], on_update=[]),
                                bass_nofuse=True,
                            )
                        )
                        n += 1
                    inst.sync_info = mybir.SyncInfo(
                        on_wait=[waits[-1]], on_update=list(si.on_update)
                    )
                    changed = True
                new.append(inst)
            if changed:
                bb.instructions[:] = new
    return n


def _build():
    if "nc" in _cache:
        return _cache["nc"]
    import concourse.bass as bass
    import concourse.mybir as mybir
    import concourse.tile as tile

    f32 = mybir.dt.float32
    bf16 = mybir.dt.bfloat16
    AF = mybir.ActivationFunctionType
    MUL = mybir.AluOpType.mult

    nc = bass.Bass("TRN2", debug=False)

    xT_d = nc.dram_tensor("xT", [H, S], bf16, kind="ExternalInput")
    initT_d = nc.dram_tensor("initT", [H, M], bf16, kind="ExternalInput")
    wqT_d = nc.dram_tensor("wqT", [H, H], bf16, kind="ExternalInput")
    wkT_d = nc.dram_tensor("wkT", [H, H], bf16, kind="ExternalInput")
    wvT_d = nc.dram_tensor("wvT", [H, H], bf16, kind="ExternalInput")
    wo1T_d = nc.dram_tensor("wo1T", [H, H], bf16, kind="ExternalInput")
    wo2T_d = nc.dram_tensor("wo2T", [H, H], bf16, kind="ExternalInput")
    bq_d = nc.dram_tensor("bqt", [P, KT], f32, kind="ExternalInput")
    bk_d = nc.dram_tensor("bkt", [P, KT], f32, kind="ExternalInput")
    bv_d = nc.dram_tensor("bvt", [P, KT], f32, kind="ExternalInput")
    bo_d = nc.dram_tensor("bot", [P, KT], f32, kind="ExternalInput")
    outT_d = nc.dram_tensor("outT", [H, S], f32, kind="ExternalOutput")

    with tile.TileContext(nc) as tc:
        const_cm = tc.tile_pool(name="const", bufs=1)
        const = const_cm.__enter__()
        wq_sb = const.tile([P, KT, H], bf16, name="wq_sb")
        wo1_sb = const.tile([P, KT, H], bf16, name="wo1_sb")
        kn_sb = const.tile([P, KT, M], bf16, name="kn_sb")    # knT: [h' part, m]
        v2_sb = const.tile([P, KT, H], bf16, name="v2_sb")    # V2: [m part, h']
        ones_sb = const.tile([P, 1], bf16, name="ones_sb")
        onesrow_sb = const.tile([1, P], f32, name="onesrow_sb")
        bq_sb = const.tile([P, KT], f32, name="bq_sb")
        bk_sb = const.tile([P, KT], f32, name="bk_sb")
        bv_sb = const.tile([P, KT], f32, name="bv_sb")
        bo_sb = const.tile([P, KT], f32, name="bo_sb")

        nc.vector.memset(ones_sb[:], 1.0)
        nc.vector.memset(onesrow_sb[:], 1.0)
        nc.sync.dma_start(out=bq_sb[:], in_=bq_d.ap())
        nc.sync.dma_start(out=bk_sb[:], in_=bk_d.ap())
        nc.sync.dma_start(out=bv_sb[:], in_=bv_d.ap())
        nc.sync.dma_start(out=bo_sb[:], in_=bo_d.ap())
        for kt in range(KT):
            nc.sync.dma_start(out=wq_sb[:, kt, :], in_=wqT_d.ap()[kt * P:(kt + 1) * P, :])
            nc.sync.dma_start(out=wo1_sb[:, kt, :], in_=wo1T_d.ap()[kt * P:(kt + 1) * P, :])

        psA_cm = tc.tile_pool(name="psA", bufs=4, space="PSUM")
        psA = psA_cm.__enter__()
        psB_cm = tc.tile_pool(name="psB", bufs=2, space="PSUM")
        psB = psB_cm.__enter__()

        # ---------- phase 0: memory init ----------
        with (
            tc.tile_pool(name="ph0", bufs=1) as ph0,
            tc.tile_pool(name="ph0tmp", bufs=3) as ph0tmp,
        ):
            wk_sb = ph0.tile([P, KT, H], bf16, name="wk_sb")
            wv_sb = ph0.tile([P, KT, H], bf16, name="wv_sb")
            wo2_sb = ph0.tile([P, KT, H], bf16, name="wo2_sb")
            init_sb = ph0.tile([P, KT, M], bf16, name="init_sb")
            vm_sb = ph0.tile([P, KT, M], bf16, name="vm_sb")  # VmemT: [h part, m]
            for kt in range(KT):
                nc.sync.dma_start(out=wk_sb[:, kt, :], in_=wkT_d.ap()[kt * P:(kt + 1) * P, :])
                nc.sync.dma_start(out=wv_sb[:, kt, :], in_=wvT_d.ap()[kt * P:(kt + 1) * P, :])
                nc.sync.dma_start(out=wo2_sb[:, kt, :], in_=wo2T_d.ap()[kt * P:(kt + 1) * P, :])
                nc.sync.dma_start(out=init_sb[:, kt, :], in_=initT_d.ap()[kt * P:(kt + 1) * P, :])

            # memory keys + column norms (norm over h' = partition dim via
            # PE ones-reduce on squared tiles), then normalize in place
            for mh in range(M // SC):
                ms = slice(mh * SC, (mh + 1) * SC)
                kn2_ps = psB.tile([1, SC], f32, name="kn2_ps", tag="acc")
                for ht in range(KT):
                    ps = psA.tile([P, SC], f32, name="kps", tag="mm")
                    for kt in range(KT):
                        nc.tensor.matmul(
                            ps[:],
                            wk_sb[:, kt, ht * P:(ht + 1) * P],
                            init_sb[:, kt, ms],
                            start=(kt == 0),
                            stop=(kt == KT - 1),
                        )
                    nc.scalar.activation(
                        out=kn_sb[:, ht, ms], in_=ps[:], func=AF.Identity,
                        bias=bk_sb[:, ht:ht + 1],
                    )
                    k2 = ph0tmp.tile([P, SC], bf16, name="k2")
                    nc.scalar.activation(
                        out=k2[:], in_=ps[:], func=AF.Square,
                        bias=bk_sb[:, ht:ht + 1],
                    )
                    nc.tensor.matmul(
                        kn2_ps[:], ones_sb[:], k2[:],
                        start=(ht == 0), stop=(ht == KT - 1),
                    )
                knorm = ph0tmp.tile([1, SC], f32, name="knorm")
                nc.scalar.activation(out=knorm[:], in_=kn2_ps[:], func=AF.Sqrt)
                kscale = ph0tmp.tile([1, SC], f32, name="kscale")
                nc.vector.reciprocal(out=kscale[:], in_=knorm[:])
                kscale_b = psA.tile([P, SC], f32, name="kscale_b", tag="mm")
                nc.tensor.matmul(kscale_b[:], onesrow_sb[:], kscale[:], start=True, stop=True)
                for ht in range(KT):
                    nc.vector.tensor_tensor(
                        out=kn_sb[:, ht, ms], in0=kn_sb[:, ht, ms],
                        in1=kscale_b[:], op=MUL,
                    )

            # memory values (VmemT), then V2[m, h'] = Vmem @ Wo2^T
            for mh in range(M // SC):
                ms = slice(mh * SC, (mh + 1) * SC)
                for ht in range(KT):
                    ps = psA.tile([P, SC], f32, name="vps", tag="mm")
                    for kt in range(KT):
                        nc.tensor.matmul(
                            ps[:],
                            wv_sb[:, kt, ht * P:(ht + 1) * P],
                            init_sb[:, kt, ms],
                            start=(kt == 0),
                            stop=(kt == KT - 1),
                        )
                    nc.scalar.activation(
                        out=vm_sb[:, ht, ms], in_=ps[:], func=AF.Identity,
                        bias=bv_sb[:, ht:ht + 1],
                    )
            for mt in range(MT):
                for hh in range(H // SC):
                    hs = slice(hh * SC, (hh + 1) * SC)
                    ps = psA.tile([P, SC], f32, name="v2ps", tag="mm")
                    for kt in range(KT):
                        nc.tensor.matmul(
                            ps[:],
                            vm_sb[:, kt, mt * P:(mt + 1) * P],
                            wo2_sb[:, kt, hs],
                            start=(kt == 0),
                            stop=(kt == KT - 1),
                        )
                    nc.scalar.activation(out=v2_sb[:, mt, hs], in_=ps[:], func=AF.Copy)

        # ---------- phase 1: per s-chunk ----------
        with (
            tc.tile_pool(name="xq", bufs=2) as xq,
            tc.tile_pool(name="mid", bufs=2) as mid,
            tc.tile_pool(name="outp", bufs=2) as outp,
            tc.tile_pool(name="small", bufs=2) as small,
        ):
            for ch in range(NCH):
                cs = slice(ch * SC, (ch + 1) * SC)
                x_sb = xq.tile([P, KT, SC], bf16, name="x_sb")
                for kt in range(KT):
                    nc.sync.dma_start(out=x_sb[:, kt, :], in_=xT_d.ap()[kt * P:(kt + 1) * P, cs])

                # Q projection + column norms
                qn = xq.tile([P, KT, SC], bf16, name="qn")
                qn2_ps = psB.tile([1, SC], f32, name="qn2_ps", tag="acc")
                for ht in range(KT):
                    ps = psA.tile([P, SC], f32, name="qps", tag="mm")
                    for kt in range(KT):
                        nc.tensor.matmul(
                            ps[:],
                            wq_sb[:, kt, ht * P:(ht + 1) * P],
                            x_sb[:, kt, :],
                            start=(kt == 0),
                            stop=(kt == KT - 1),
                        )
                    nc.scalar.activation(
                        out=qn[:, ht, :], in_=ps[:], func=AF.Identity,
                        bias=bq_sb[:, ht:ht + 1],
                    )
                    q2 = mid.tile([P, SC], bf16, name="q2")
                    nc.scalar.activation(
                        out=q2[:], in_=ps[:], func=AF.Square,
                        bias=bq_sb[:, ht:ht + 1],
                    )
                    nc.tensor.matmul(
                        qn2_ps[:], ones_sb[:], q2[:],
                        start=(ht == 0), stop=(ht == KT - 1),
                    )
                qnorm = small.tile([1, SC], f32, name="qnorm")
                nc.scalar.activation(out=qnorm[:], in_=qn2_ps[:], func=AF.Sqrt)
                qscale = small.tile([1, SC], f32, name="qscale")
                nc.vector.reciprocal(out=qscale[:], in_=qnorm[:])
                qscale_b = psA.tile([P, SC], f32, name="qscale_b", tag="mm")
                nc.tensor.matmul(qscale_b[:], onesrow_sb[:], qscale[:], start=True, stop=True)
                for ht in range(KT):
                    nc.vector.tensor_tensor(
                        out=qn[:, ht, :], in0=qn[:, ht, :], in1=qscale_b[:], op=MUL,
                    )

                # cosine sims -> exp -> softmax weights (attnT, in place on expT)
                expT = xq.tile([P, MT, SC], bf16, name="expT")
                se_ps = psB.tile([1, SC], f32, name="se_ps", tag="acc")
                for mt in range(MT):
                    ps = psA.tile([P, SC], f32, name="sps", tag="mm")
                    for ht in range(KT):
                        nc.tensor.matmul(
                            ps[:],
                            kn_sb[:, ht, mt * P:(mt + 1) * P],
                            qn[:, ht, :],
                            start=(ht == 0),
                            stop=(ht == KT - 1),
                        )
                    nc.scalar.activation(out=expT[:, mt, :], in_=ps[:], func=AF.Exp)
                    nc.tensor.matmul(
                        se_ps[:], ones_sb[:], expT[:, mt, :],
                        start=(mt == 0), stop=(mt == MT - 1),
                    )
                rsum = small.tile([1, SC], f32, name="rsum")
                nc.vector.reciprocal(out=rsum[:], in_=se_ps[:])
                rsum_b = psA.tile([P, SC], f32, name="rsum_b", tag="mm")
                nc.tensor.matmul(rsum_b[:], onesrow_sb[:], rsum[:], start=True, stop=True)
                for mt in range(MT):
                    nc.vector.tensor_tensor(
                        out=expT[:, mt, :], in0=expT[:, mt, :], in1=rsum_b[:], op=MUL,
                    )

                # out = Wo1 @ xT + V2^T @ attnT + bo, single PSUM chain
                o_sb = outp.tile([P, KT, SC], f32, name="o_sb")
                for ht in range(KT):
                    ps = psA.tile([P, SC], f32, name="ops", tag="mm")
                    for kt in range(KT):
                        nc.tensor.matmul(
                            ps[:],
                            wo1_sb[:, kt, ht * P:(ht + 1) * P],
                            x_sb[:, kt, :],
                            start=(kt == 0),
                            stop=False,
                        )
                    for mt in range(MT):
                        nc.tensor.matmul(
                            ps[:],
                            v2_sb[:, mt, ht * P:(ht + 1) * P],
                            expT[:, mt, :],
                            start=False,
                            stop=(mt == MT - 1),
                        )
                    nc.scalar.activation(
                        out=o_sb[:, ht, :], in_=ps[:], func=AF.Identity,
                        bias=bo_sb[:, ht:ht + 1],
                    )
                    nc.sync.dma_start(out=outT_d.ap()[ht * P:(ht + 1) * P, cs], in_=o_sb[:, ht, :])
        psB_cm.__exit__(None, None, None)
        psA_cm.__exit__(None, None, None)
        const_cm.__exit__(None, None, None)

    _split_excess_waits(nc, mybir)
    _cache["nc"] = nc
    return nc


def kernel(hidden_states, Wq, bq, Wk, bk, Wv, bv, Wo, bo):
    from concourse import bass_utils

    hidden_states = np.asarray(hidden_states, dtype=np.float32)
    idx = _idx()

    wqT = np.ascontiguousarray(np.asarray(Wq, np.float32).T).astype(BF16)
    wkT = np.ascontiguousarray(np.asarray(Wk, np.float32).T).astype(BF16)
    wvT = np.ascontiguousarray(np.asarray(Wv, np.float32).T).astype(BF16)
    Wo = np.asarray(Wo, np.float32)
    wo1T = np.ascontiguousarray(Wo[:, :H].T).astype(BF16)
    wo2T = np.ascontiguousarray(Wo[:, H:].T).astype(BF16)

    def btile(b):
        return np.ascontiguousarray(np.asarray(b, np.float32).reshape(KT, P).T)

    bqt, bkt, bvt, bot = btile(bq), btile(bk), btile(bv), btile(bo)

    in_maps = []
    for b in range(B):
        xT = np.ascontiguousarray(hidden_states[b].T).astype(BF16)
        in_maps.append({
            "xT": xT,
            "initT": np.ascontiguousarray(xT[:, idx]),
            "wqT": wqT, "wkT": wkT, "wvT": wvT, "wo1T": wo1T, "wo2T": wo2T,
            "bqt": bqt, "bkt": bkt, "bvt": bvt, "bot": bot,
        })

    nc = _build()
    res = bass_utils.run_bass_kernel_spmd(nc, in_maps, core_ids=list(range(N_CORES)))

    out = np.empty((B, S, H), np.float32)
    for b in range(B):
        out[b] = res.results[b]["outT"].T
    return out


# revision 11
# speedup vs baseline: 101155.4800x; 101155.4800x over previous
"""Trainium2 Bass kernel for DifferentiableMemory (B=8, S=4096, H=1024, M=1024).

Data-parallel over batch: one batch per NeuronCore x 8 cores, weights
replicated. All on-device activations live feature-on-partition ("T layout"),
produced by free host-side transposes, so no on-device transposes are needed:

  per core (batch b), with xT = hidden[b].T as [H, S] bf16:
    KmemT = WkT-proj(initT)     initT = xT[:, idx] (static gather, host-side)
    kn    = KmemT * rsqrt(colsum(KmemT^2))      (PE ones-reduce over partitions)
    V2    = init @ Wvo^T        Wvo = Wo2 @ Wv host-precomputed: folds
                                (attn @ Vmem) @ Wo2^T into attn @ V2
    per 512-col chunk of s:
      QT    = WqT-proj(xT chunk); qn = QT * rsqrt(colsum(QT^2))
      simT  = kn^T-matmul(qn)    [m, s]  (cosine sims, bounded -> exp w/o max)
      expT  = Exp(simT); attnT = expT * recip(ones-reduce(expT))
      outT  = Wo1-proj(xT chunk) + V2-matmul(attnT) + bo   (single PSUM chain)

Matmuls in bf16 (fp32 PSUM accumulation); output returned fp32. Row-scale
broadcasts ([1,N] -> [128,N]) are done with a K=1 PE matmul against a ones
row since this walrus build rejects InstPartitionBroadcast.
"""
import sys

sys.path.insert(0, "/opt/trn_rl_repo")

import numpy as np
import ml_dtypes

BF16 = ml_dtypes.bfloat16

B, S, H, M = 8, 4096, 1024, 1024
N_CORES = 8
P = 128          # partitions
KT = H // P      # 8 feature tiles
SC = 512         # s-chunk (PSUM bank = 512 fp32)
NCH = S // SC    # 8 chunks
MT = M // P      # 8 memory tiles

_cache = {}


def _idx():
    """Replicate reference: jnp.linspace(0.0, s-1, M).astype(int32).
    Computed with in-process jax so platform-specific fp32 rounding matches
    the grader's reference; numpy fallback differs in at most a few slots."""
    if "idx" in _cache:
        return _cache["idx"]
    try:
        import jax.numpy as jnp

        idx = np.asarray(jnp.linspace(0.0, S - 1, M).astype(jnp.int32))
    except Exception:
        idx = np.linspace(0.0, S - 1, M).astype(np.float32).astype(np.int32)
    _cache["idx"] = idx
    return idx


def _split_excess_waits(nc, mybir):
    """This container's walrus accepts at most 1 sem-wait per instruction
    (setupSyncWait raises "Too many sync wait commands" beyond that), while
    Tile's add_semaphores freely attaches several. Move excess waits onto
    preceding same-engine NoOps - engine streams execute in order, so a wait
    on an earlier instruction gates everything after it."""
    n = 0
    for f in nc.m.functions:
        for bb in f.blocks:
            insts = list(bb.instructions)
            new = []
            changed = False
            for inst in insts:
                si = getattr(inst, "sync_info", None)
                waits = list(si.on_wait) if si is not None and si.on_wait else []
                if len(waits) > 1:
                    for w in waits[:-1]:
                        new.append(
                            mybir.InstNoOp(
                                name=f"{inst.name}-wsplit{n}",
                                engine=inst.engine,
                                sync_info=mybir.SyncInfo(on_wait=[w], on_update=[]),
                                bass_nofuse=True,
                            )
                        )
                        n += 1
                    inst.sync_info = mybir.SyncInfo(
                        on_wait=[waits[-1]], on_update=list(si.on_update)
                    )
                    changed = True
                new.append(inst)
            if changed:
                bb.instructions[:] = new
    return n


def _build(**opts):
    key = ("nc", tuple(sorted(opts.items())))
    if key in _cache:
        return _cache[key]
    psa_bufs = opts.get("psa_bufs", 6)
    psb_bufs = opts.get("psb_bufs", 2)
    xq_bufs = opts.get("xq_bufs", 2)
    mid_bufs = opts.get("mid_bufs", 2)
    outp_bufs = opts.get("outp_bufs", 3)
    small_bufs = opts.get("small_bufs", 2)
    ph0tmp_bufs = opts.get("ph0tmp_bufs", 3)
    nch = opts.get("nch", NCH)
    skip_ph0 = opts.get("skip_ph0", False)

    import concourse.bass as bass
    import concourse.mybir as mybir
    import concourse.tile as tile

    f32 = mybir.dt.float32
    bf16 = mybir.dt.bfloat16
    AF = mybir.ActivationFunctionType
    MUL = mybir.AluOpType.mult

    nc = bass.Bass("TRN2", debug=False)

    xT_d = nc.dram_tensor("xT", [H, S], bf16, kind="ExternalInput")
    initT_d = nc.dram_tensor("initT", [H, M], bf16, kind="ExternalInput")
    wqT_d = nc.dram_tensor("wqT", [H, H], bf16, kind="ExternalInput")
    wkT_d = nc.dram_tensor("wkT", [H, H], bf16, kind="ExternalInput")
    wvoT_d = nc.dram_tensor("wvoT", [H, H], bf16, kind="ExternalInput")
    wo1T_d = nc.dram_tensor("wo1T", [H, H], bf16, kind="ExternalInput")
    bq_d = nc.dram_tensor("bqt", [P, KT], f32, kind="ExternalInput")
    bk_d = nc.dram_tensor("bkt", [P, KT], f32, kind="ExternalInput")
    bo_d = nc.dram_tensor("bot", [P, KT], f32, kind="ExternalInput")
    outT_d = nc.dram_tensor("outT", [H, S], f32, kind="ExternalOutput")

    with tile.TileContext(nc) as tc:
        with (
            tc.tile_pool(name="const", bufs=1) as const,
            tc.tile_pool(name="ph0", bufs=1) as ph0,
            tc.tile_pool(name="ph0tmp", bufs=ph0tmp_bufs) as ph0tmp,
            tc.tile_pool(name="xq", bufs=xq_bufs) as xq,
            tc.tile_pool(name="mid", bufs=mid_bufs) as mid,
            tc.tile_pool(name="outp", bufs=outp_bufs) as outp,
            tc.tile_pool(name="small", bufs=small_bufs) as small,
            tc.tile_pool(name="psA", bufs=psa_bufs, space="PSUM") as psA,
            tc.tile_pool(name="psB", bufs=psb_bufs, space="PSUM") as psB,
        ):
            wq_sb = const.tile([P, KT, H], bf16, name="wq_sb")
            wo1_sb = const.tile([P, KT, H], bf16, name="wo1_sb")
            kn_sb = const.tile([P, KT, M], bf16, name="kn_sb")   # knT: [h' part, m]
            v2_sb = const.tile([P, KT, H], bf16, name="v2_sb")   # V2: [m part, h']
            ones_sb = const.tile([P, 1], bf16, name="ones_sb")
            onesrow_sb = const.tile([1, P], f32, name="onesrow_sb")
            bq_sb = const.tile([P, KT], f32, name="bq_sb")
            bk_sb = const.tile([P, KT], f32, name="bk_sb")
            bo_sb = const.tile([P, KT], f32, name="bo_sb")

            nc.vector.memset(ones_sb[:], 1.0)
            nc.vector.memset(onesrow_sb[:], 1.0)
            nc.sync.dma_start(out=bq_sb[:], in_=bq_d.ap())
            nc.sync.dma_start(out=bk_sb[:], in_=bk_d.ap())
            nc.sync.dma_start(out=bo_sb[:], in_=bo_d.ap())
            for kt in range(KT):
                nc.sync.dma_start(out=wq_sb[:, kt, :], in_=wqT_d.ap()[kt * P:(kt + 1) * P, :])
                nc.sync.dma_start(out=wo1_sb[:, kt, :], in_=wo1T_d.ap()[kt * P:(kt + 1) * P, :])

            # ---------- phase 0: memory init (overlaps with early Q chunks) ----
            wk_sb = ph0.tile([P, KT, H], bf16, name="wk_sb")
            wvo_sb = ph0.tile([P, KT, H], bf16, name="wvo_sb")
            init_sb = ph0.tile([P, KT, M], bf16, name="init_sb")
            for kt in range(KT):
                nc.sync.dma_start(out=wk_sb[:, kt, :], in_=wkT_d.ap()[kt * P:(kt + 1) * P, :])
                nc.sync.dma_start(out=wvo_sb[:, kt, :], in_=wvoT_d.ap()[kt * P:(kt + 1) * P, :])
                nc.sync.dma_start(out=init_sb[:, kt, :], in_=initT_d.ap()[kt * P:(kt + 1) * P, :])

            # memory keys + column norms (norm over h' = partition dim via PE
            # ones-reduce on squared tiles), then normalize in place
            for mh in range(0 if skip_ph0 else (M // SC)):
                ms = slice(mh * SC, (mh + 1) * SC)
                kn2_ps = psB.tile([1, SC], f32, name="kn2_ps", tag="acc")
                for ht in range(KT):
                    ps = psA.tile([P, SC], f32, name="kps", tag="mm")
                    for kt in range(KT):
                        nc.tensor.matmul(
                            ps[:],
                            wk_sb[:, kt, ht * P:(ht + 1) * P],
                            init_sb[:, kt, ms],
                            start=(kt == 0),
                            stop=(kt == KT - 1),
                        )
                    nc.scalar.activation(
                        out=kn_sb[:, ht, ms], in_=ps[:], func=AF.Identity,
                        bias=bk_sb[:, ht:ht + 1],
                    )
                    k2 = ph0tmp.tile([P, SC], bf16, name="k2")
                    nc.scalar.activation(
                        out=k2[:], in_=ps[:], func=AF.Square,
                        bias=bk_sb[:, ht:ht + 1],
                    )
                    nc.tensor.matmul(
                        kn2_ps[:], ones_sb[:], k2[:],
                        start=(ht == 0), stop=(ht == KT - 1),
                    )
                knorm = ph0tmp.tile([1, SC], f32, name="knorm")
                nc.scalar.activation(out=knorm[:], in_=kn2_ps[:], func=AF.Sqrt)
                kscale = ph0tmp.tile([1, SC], f32, name="kscale")
                nc.vector.reciprocal(out=kscale[:], in_=knorm[:])
                kscale_b = psA.tile([P, SC], f32, name="kscale_b", tag="mm")
                nc.tensor.matmul(kscale_b[:], onesrow_sb[:], kscale[:], start=True, stop=True)
                for ht in range(KT):
                    nc.vector.tensor_tensor(
                        out=kn_sb[:, ht, ms], in0=kn_sb[:, ht, ms],
                        in1=kscale_b[:], op=MUL,
                    )

            # V2[m, h'] = init @ Wvo^T  (Wvo = Wo2 @ Wv, host-precomputed)
            for mt in range(0 if skip_ph0 else MT):
                for hh in range(H // SC):
                    hs = slice(hh * SC, (hh + 1) * SC)
                    ps = psA.tile([P, SC], f32, name="v2ps", tag="mm")
                    for kt in range(KT):
                        nc.tensor.matmul(
                            ps[:],
                            init_sb[:, kt, mt * P:(mt + 1) * P],
                            wvo_sb[:, kt, hs],
                            start=(kt == 0),
                            stop=(kt == KT - 1),
                        )
                    nc.scalar.activation(out=v2_sb[:, mt, hs], in_=ps[:], func=AF.Copy)

            # ---------- phase 1: per s-chunk ----------
            for ch in range(nch):
                cs = slice(ch * SC, (ch + 1) * SC)
                x_sb = xq.tile([P, KT, SC], bf16, name="x_sb")
                for kt in range(KT):
                    nc.sync.dma_start(out=x_sb[:, kt, :], in_=xT_d.ap()[kt * P:(kt + 1) * P, cs])

                # Q projection + column norms
                qn = xq.tile([P, KT, SC], bf16, name="qn")
                qn2_ps = psB.tile([1, SC], f32, name="qn2_ps", tag="acc")
                for ht in range(KT):
                    ps = psA.tile([P, SC], f32, name="qps", tag="mm")
                    for kt in range(KT):
                        nc.tensor.matmul(
                            ps[:],
                            wq_sb[:, kt, ht * P:(ht + 1) * P],
                            x_sb[:, kt, :],
                            start=(kt == 0),
                            stop=(kt == KT - 1),
                        )
                    nc.scalar.activation(
                        out=qn[:, ht, :], in_=ps[:], func=AF.Identity,
                        bias=bq_sb[:, ht:ht + 1],
                    )
                    q2 = mid.tile([P, SC], bf16, name="q2")
                    nc.scalar.activation(
                        out=q2[:], in_=ps[:], func=AF.Square,
                        bias=bq_sb[:, ht:ht + 1],
                    )
                    nc.tensor.matmul(
                        qn2_ps[:], ones_sb[:], q2[:],
                        start=(ht == 0), stop=(ht == KT - 1),
                    )
                qnorm = small.tile([1, SC], f32, name="qnorm")
                nc.scalar.activation(out=qnorm[:], in_=qn2_ps[:], func=AF.Sqrt)
                qscale = small.tile([1, SC], f32, name="qscale")
                nc.vector.reciprocal(out=qscale[:], in_=qnorm[:])
                qscale_b = psA.tile([P, SC], f32, name="qscale_b", tag="mm")
                nc.tensor.matmul(qscale_b[:], onesrow_sb[:], qscale[:], start=True, stop=True)
                for ht in range(KT):
                    nc.vector.tensor_tensor(
                        out=qn[:, ht, :], in0=qn[:, ht, :], in1=qscale_b[:], op=MUL,
                    )

                # cosine sims -> exp -> softmax weights (in place on expT)
                expT = xq.tile([P, MT, SC], bf16, name="expT")
                se_ps = psB.tile([1, SC], f32, name="se_ps", tag="acc")
                for mt in range(MT):
                    ps = psA.tile([P, SC], f32, name="sps", tag="mm")
                    for ht in range(KT):
                        nc.tensor.matmul(
                            ps[:],
                            kn_sb[:, ht, mt * P:(mt + 1) * P],
                            qn[:, ht, :],
                            start=(ht == 0),
                            stop=(ht == KT - 1),
                        )
                    nc.scalar.activation(out=expT[:, mt, :], in_=ps[:], func=AF.Exp)
                    nc.tensor.matmul(
                        se_ps[:], ones_sb[:], expT[:, mt, :],
                        start=(mt == 0), stop=(mt == MT - 1),
                    )
                rsum = small.tile([1, SC], f32, name="rsum")
                nc.vector.reciprocal(out=rsum[:], in_=se_ps[:])
                rsum_b = psA.tile([P, SC], f32, name="rsum_b", tag="mm")
                nc.tensor.matmul(rsum_b[:], onesrow_sb[:], rsum[:], start=True, stop=True)
                for mt in range(MT):
                    nc.vector.tensor_tensor(
                        out=expT[:, mt, :], in0=expT[:, mt, :], in1=rsum_b[:], op=MUL,
                    )

                # out = Wo1 @ xT + V2^T @ attnT + bo, single PSUM chain per tile
                for ht in range(KT):
                    ps = psA.tile([P, SC], f32, name="ops", tag="mm")
                    for kt in range(KT):
                        nc.tensor.matmul(
                            ps[:],
                            wo1_sb[:, kt, ht * P:(ht + 1) * P],
                            x_sb[:, kt, :],
                            start=(kt == 0),
                            stop=False,
                        )
                    for mt in range(MT):
                        nc.tensor.matmul(
                            ps[:],
                            v2_sb[:, mt, ht * P:(ht + 1) * P],
                            expT[:, mt, :],
                            start=False,
                            stop=(mt == MT - 1),
                        )
                    o_sb = outp.tile([P, SC], f32, name="o_sb")
                    nc.scalar.activation(
                        out=o_sb[:], in_=ps[:], func=AF.Identity,
                        bias=bo_sb[:, ht:ht + 1],
                    )
                    nc.sync.dma_start(out=outT_d.ap()[ht * P:(ht + 1) * P, cs], in_=o_sb[:])

    _split_excess_waits(nc, mybir)
    _cache[key] = nc
    return nc


def _prep_inputs(hidden_states, Wq, bq, Wk, bk, Wv, bv, Wo, bo):
    hidden_states = np.asarray(hidden_states, dtype=np.float32)
    idx = _idx()

    Wq = np.asarray(Wq, np.float32)
    Wk = np.asarray(Wk, np.float32)
    Wv = np.asarray(Wv, np.float32)
    Wo = np.asarray(Wo, np.float32)
    bv = np.asarray(bv, np.float32)
    bo = np.asarray(bo, np.float32)
    Wo2 = Wo[:, H:]
    wqT = np.ascontiguousarray(Wq.T).astype(BF16)
    wkT = np.ascontiguousarray(Wk.T).astype(BF16)
    wo1T = np.ascontiguousarray(Wo[:, :H].T).astype(BF16)
    # fold (attn @ (init@Wv^T + bv)) @ Wo2^T into attn @ (init@Wvo^T) + bvo;
    # softmax rows sum to 1, so the bv term lands as a constant bvo in bo
    wvoT = np.ascontiguousarray((Wo2 @ Wv).T).astype(BF16)
    bo_eff = bo + Wo2 @ bv

    def btile(b):
        return np.ascontiguousarray(np.asarray(b, np.float32).reshape(KT, P).T)

    bqt, bkt, bot = btile(bq), btile(bk), btile(bo_eff)

    in_maps = []
    for b in range(B):
        xT = np.ascontiguousarray(hidden_states[b].T).astype(BF16)
        in_maps.append({
            "xT": xT,
            "initT": np.ascontiguousarray(xT[:, idx]),
            "wqT": wqT, "wkT": wkT, "wvoT": wvoT, "wo1T": wo1T,
            "bqt": bqt, "bkt": bkt, "bot": bot,
        })
    return in_maps


def kernel(hidden_states, Wq, bq, Wk, bk, Wv, bv, Wo, bo):
    from concourse import bass_utils

    in_maps = _prep_inputs(hidden_states, Wq, bq, Wk, bk, Wv, bv, Wo, bo)
    nc = _build()
    res = bass_utils.run_bass_kernel_spmd(nc, in_maps, core_ids=list(range(N_CORES)))

    out = np.empty((B, S, H), np.float32)
    for b in range(B):
        out[b] = res.results[b]["outT"].T
    return out
